# revision 13
# baseline (speedup 1.0000x reference)
"""Multi-head attention (B=16, C=256, N=1024, H=4 heads) on 8 TRN2 NeuronCores.

Data-parallel over batch: 2 images per core, weights replicated, no
collectives. All five GEMM stages (qkv proj, scores, softmax denominator,
AV, out proj) run in fp8 e4m3 with DoubleRow perf mode -- each matmul
contracts 256 rows (2 fp8 weights/cell) in 512 cycles, ~2x the bf16 rate.
fp32 PSUM accumulation throughout; simulated end-to-end rel err ~8e-3
(tolerance 2e-2). Softmax statistics and the residual stay >= bf16.

Layout strategy: everything stays "transposed" ([feature, token]) so the
whole chain needs zero on-chip transposes:
  qk8[4, N]   = W_proj_slices.T @ x8    (DR: lhsT = w8qk [ci,kt,*], rhs = x8)
  attT[j, i]  = k8 @ q8.T               (DR: lhsT/rhs = qk8 slot pairs)
  E8          = exp(attT*scale - ln32)  (ScalarE, PSUM -> e4m3 SBUF direct)
  o[d, i]     = v8.T @ E8   (DR, 4 chunks of 256 j) ; s = ones8.T @ E8
  res[c, i]   = wo8.T @ cat8 (DR) + eye_bf16 @ x_bf16  (residual folded
                into the same PSUM group; drained on ScalarE with bias)

Scheduling: engines execute their instruction streams IN ORDER, so the
emission is a software pipeline. Scores matmuls (whose PSUM tiles are
drained by ScalarE exp at ~580ns vs ~300ns/MM production) are woven with
dependency-ready "filler" matmuls pulled from a FIFO: v-proj, next head's
qk proj, previous (head, i-half)'s AV+denominator chain, out-proj.
Per-phase PSUM pools (scores / proj / AV-o / AV-s) keep ring-allocation
waits from coupling unrelated phases.

Engine budget per core (model): PE ~140us of matmuls; DVE ~94us (qk/v
PSUM drains with per-partition bias, softmax reciprocal + normalize);
ScalarE ~79us (exp over the 2x 4M-element attention matrix + final
drains); GPSIMD ~20us (all SBUF->SBUF fp8/bf16 casts). E is scaled by
1/32 inside the exp bias so e4m3 never saturates; the scale cancels
between numerator o and denominator s.

The identity matrix for the residual matmul rides in as an extra DRAM
input supplied by kernel() (np.eye), cast to bf16 on chip. b_proj's q/k
biases are applied on the qk drains (DVE tensor_scalar add); b_v folds
through softmax (weights sum to 1) into total_bias = b_out + b_v @ W_out
computed with tiny fp8 matmuls, applied at the final ScalarE drain.
"""
import sys

try:
    import concourse.bass as bass  # noqa: F401
except ImportError:
    sys.path.insert(0, "/opt/trn_rl_repo")

import math
from collections import deque
from contextlib import ExitStack

import numpy as np

import concourse.bass as bass
import concourse.mybir as mybir
import concourse.tile as tile
from concourse import bacc
from concourse.bass_utils import run_bass_kernel_spmd

F32 = mybir.dt.float32
BF16 = mybir.dt.bfloat16
E4 = mybir.dt.float8e4
EXP = mybir.ActivationFunctionType.Exp
IDENT = mybir.ActivationFunctionType.Identity
DR = mybir.MatmulPerfMode.DoubleRow
MUL = mybir.AluOpType.mult

B_PER_CORE = 2   # 16 images / 8 cores
C = 256          # channels == head dim
N = 1024         # tokens (32*32)
HEADS = 4
SCALE = C ** -0.5
N_CORES = 8
NLOG32 = -math.log(32.0)


def _build():
    nc = bacc.Bacc("TRN2", debug=False, num_devices=N_CORES)
    x_d = nc.declare_dram_parameter("x", [B_PER_CORE, C, N], F32, isOutput=False)
    wp_d = nc.declare_dram_parameter("W_proj", [C, 3 * HEADS * C], F32, isOutput=False)
    bp_d = nc.declare_dram_parameter("b_proj", [3 * HEADS * C], F32, isOutput=False)
    wo_d = nc.declare_dram_parameter("W_out", [HEADS * C, C], F32, isOutput=False)
    bo_d = nc.declare_dram_parameter("b_out", [C], F32, isOutput=False)
    eye_d = nc.declare_dram_parameter("eye", [128, 128], F32, isOutput=False)
    out_d = nc.declare_dram_parameter("out", [B_PER_CORE, C, N], F32, isOutput=True)

    with tile.TileContext(nc) as tc, ExitStack() as ctx:
        pool = ctx.enter_context(tc.tile_pool(name="persist", bufs=1))
        stage_pool = ctx.enter_context(tc.tile_pool(name="stage", bufs=6))
        xr_pool = ctx.enter_context(tc.tile_pool(name="xr", bufs=2))
        x8_pool = ctx.enter_context(tc.tile_pool(name="x8", bufs=2))
        xb_pool = ctx.enter_context(tc.tile_pool(name="xb", bufs=2))
        qk_pool = ctx.enter_context(tc.tile_pool(name="qk", bufs=3))
        e_pool = ctx.enter_context(tc.tile_pool(name="e8", bufs=3))
        v_pool = ctx.enter_context(tc.tile_pool(name="v8", bufs=2))
        cat_pool = ctx.enter_context(tc.tile_pool(name="cat", bufs=2))
        r_pool = ctx.enter_context(tc.tile_pool(name="r", bufs=2))
        out_pool = ctx.enter_context(tc.tile_pool(name="outs", bufs=4))
        psS = ctx.enter_context(tc.tile_pool(name="psS", bufs=2, space="PSUM"))
        psQ = ctx.enter_context(tc.tile_pool(name="psQ", bufs=2, space="PSUM"))
        psB = ctx.enter_context(tc.tile_pool(name="psB", bufs=2, space="PSUM"))
        psC = ctx.enter_context(tc.tile_pool(name="psC", bufs=2, space="PSUM"))

        # ---- constants first (GPSIMD memsets): they gate the PE warmup ----
        ones_w = pool.tile([128, 512], BF16)
        nc.gpsimd.memset(ones_w[:], 1.0)
        ones8 = pool.tile([128, 2, 128], E4)
        nc.gpsimd.memset(ones8[:], 1.0)
        ebias = pool.tile([128, 1], F32)  # exp bias: -ln(32)
        nc.gpsimd.memset(ebias[:], NLOG32)

        # ---- DMAs + GPSIMD fp8 casts, first-needed data first ----
        xr_tiles = []
        # DMA issue order maps round-robin onto the HW queues, and queues come
        # online in index order at startup -- so the startup-critical bytes
        # (W_proj head 0, then x image-0) go first, with >=2KB contiguous rows
        # per transfer (small rows pay ~160ns/descriptor).
        w8qk = pool.tile([128, 2, 2048], E4)
        w8v = pool.tile([128, 2, 1024], E4)
        ws_tiles = {}
        for kt in range(2):
            ws = stage_pool.tile([128, 768], F32, tag="wstage")
            ws_tiles[(0, kt)] = ws
            nc.sync.dma_start(out=ws[:], in_=wp_d[kt * 128:(kt + 1) * 128, 0:768])
        xr = xr_pool.tile([128, 2, N], F32, tag="xr")
        for isl in range(2):
            for kt in range(2):
                nc.sync.dma_start(
                    out=xr[:, kt, isl * 512:(isl + 1) * 512],
                    in_=x_d[0, kt * 128:(kt + 1) * 128, isl * 512:(isl + 1) * 512])
        xr_tiles.append(xr)
        b_sb = pool.tile([128, 24], F32)  # b_proj, tile t
        nc.sync.dma_start(out=b_sb[:],
                          in_=bp_d[:].rearrange("(t p) -> p t", p=128))
        bo_sb = pool.tile([128, 2], F32)
        nc.sync.dma_start(out=bo_sb[:],
                          in_=bo_d[:].rearrange("(t p) -> p t", p=128))
        eye_f = stage_pool.tile([128, 128], F32, tag="eyestage")
        nc.sync.dma_start(out=eye_f[:], in_=eye_d[:, :])
        # head-0 casts ride DVE (idle this early)
        for kt in range(2):
            ws = ws_tiles[(0, kt)]
            nc.vector.tensor_copy(w8qk[:, kt, 0:512], ws[:, 0:512])
            nc.vector.tensor_copy(w8v[:, kt, 0:256], ws[:, 512:768])
        eye_bf = pool.tile([128, 128], BF16)
        nc.vector.tensor_copy(eye_bf[:], eye_f[:])

        # W_proj heads 1-3 (casts deferred until after the prologue)
        deferred_wcasts = []
        for h in range(1, HEADS):
            for kt in range(2):
                ws = stage_pool.tile([128, 768], F32, tag="wstage")
                nc.sync.dma_start(
                    out=ws[:],
                    in_=wp_d[kt * 128:(kt + 1) * 128, h * 768:(h + 1) * 768])
                deferred_wcasts.append((ws, h, kt))

        # second image's x: queued last, prefetched during image-0 compute
        xr = xr_pool.tile([128, 2, N], F32, tag="xr")
        for kt in range(2):
            for isl in range(2):
                nc.sync.dma_start(
                    out=xr[:, kt, isl * 512:(isl + 1) * 512],
                    in_=x_d[1, kt * 128:(kt + 1) * 128, isl * 512:(isl + 1) * 512])
        xr_tiles.append(xr)

        # dummy matmuls: fill the initial DMA wait + warm the HAM clock gate
        for wi in range(24):
            warm_ps = psS.tile([128, 512], F32, tag="S")
            nc.tensor.matmul(out=warm_ps[:], lhsT=ones_w[:, 0:128],
                             rhs=ones_w[:], start=True, stop=True)

        wo8 = pool.tile([128, 8, 256], E4)   # W_out k-tiles (loaded mid-image-0)
        zb = pool.tile([128, 8, 2], E4)      # b_v columns for the bias fold
        total_bias = pool.tile([128, 2], F32)

        # ---------- emission helpers (each closure emits ~one matmul) ----------
        fq = deque()

        def fpop(k):
            for _ in range(k):
                if fq:
                    fq.popleft()()

        def qk_mms(x8, qk8, h):
            """8 closures: q,k for head h -> qk8[128, slot, isl, 512] e4m3.
            Emission order matches scores' consumption order."""
            def one(mt, isl):
                def go():
                    ps = psQ.tile([128, 512], F32, tag="Q", name="ps_qk")
                    nc.tensor.matmul(
                        out=ps[:],
                        lhsT=w8qk[:, 0:2,
                                  h * 512 + mt * 128:h * 512 + (mt + 1) * 128],
                        rhs=x8[:, 0:2, isl * 512:(isl + 1) * 512],
                        perf_mode=DR, start=True, stop=True)
                    nc.vector.tensor_scalar_add(
                        qk8[:, mt, isl], ps[:],
                        b_sb[:, h * 6 + mt:h * 6 + mt + 1])
                return go
            order = [(0, 0), (1, 0), (2, 0), (3, 0), (2, 1), (3, 1), (0, 1), (1, 1)]
            return [one(mt, isl) for mt, isl in order]

        def v_mms(x8, v8):
            """16 closures: v for all heads -> v8[:, it, h*256+d]."""
            def one(it, hp):
                def go():
                    ps = psQ.tile([128, 512], F32, tag="Q", name="ps_v")
                    nc.tensor.matmul(
                        out=ps[:],
                        lhsT=x8[:, 0:2, it * 128:(it + 1) * 128],
                        rhs=w8v[:, 0:2, hp * 512:(hp + 1) * 512],
                        perf_mode=DR, start=True, stop=True)
                    nc.vector.tensor_copy(v8[:, it, hp * 512:(hp + 1) * 512],
                                          ps[:])
                return go
            return [one(it, hp) for it in range(8) for hp in range(2)]

        def av_mms(e8, v8, cat8, h, isl):
            """12 closures: AV + denominator for one i-half -> cat8 (normalized)."""
            o_ps = [None, None]
            s_ps = [None]

            def mm_o(a, dh):
                def go():
                    if o_ps[dh] is None:
                        o_ps[dh] = psB.tile([128, 512], F32, tag="B", name="o_ps")
                    nc.tensor.matmul(
                        out=o_ps[dh][:],
                        lhsT=v8[:, 2 * a:2 * a + 2,
                                h * 256 + dh * 128:h * 256 + (dh + 1) * 128],
                        rhs=e8[:, 2 * a:2 * a + 2, isl * 512:(isl + 1) * 512],
                        perf_mode=DR, start=(a == 0), stop=(a == 3))
                return go

            def mm_s(a):
                def go():
                    if s_ps[0] is None:
                        s_ps[0] = psC.tile([128, 512], F32, tag="C", name="s_ps")
                    nc.tensor.matmul(
                        out=s_ps[0][:], lhsT=ones8[:],
                        rhs=e8[:, 2 * a:2 * a + 2, isl * 512:(isl + 1) * 512],
                        perf_mode=DR, start=(a == 0), stop=(a == 3))
                    if a == 3:
                        r_sb = r_pool.tile([128, 512], F32, tag="r", name="r_sb")
                        nc.vector.reciprocal_approx_fast(r_sb[:], s_ps[0][:])
                        for dh2 in range(2):
                            nc.vector.scalar_tensor_tensor(
                                cat8[:, 2 * h + dh2, isl * 512:(isl + 1) * 512],
                                o_ps[dh2][:], 1.0, r_sb[:], MUL, MUL)
                return go

            out = []
            for a in range(4):
                out += [mm_o(a, 0), mm_o(a, 1), mm_s(a)]
            return out

        def outproj_mms(b, cat8, xb):
            """20 closures + drains + DMA: res[c, i] with residual + bias."""
            o_sb = [None, None]
            ps = {}

            def mm(ct, isl, a):
                def go():
                    if (ct, isl) not in ps:
                        ps[(ct, isl)] = psQ.tile([128, 512], F32, tag="Q", name="ps_op")
                    nc.tensor.matmul(
                        out=ps[(ct, isl)][:],
                        lhsT=wo8[:, 2 * a:2 * a + 2, ct * 128:(ct + 1) * 128],
                        rhs=cat8[:, 2 * a:2 * a + 2, isl * 512:(isl + 1) * 512],
                        perf_mode=DR, start=(a == 0), stop=False)
                return go

            def mm_eye(ct, isl):
                def go():
                    nc.tensor.matmul(out=ps[(ct, isl)][:], lhsT=eye_bf[:],
                                     rhs=xb[:, ct, isl * 512:(isl + 1) * 512],
                                     start=False, stop=True)
                    if o_sb[ct] is None:
                        o_sb[ct] = out_pool.tile([128, 1024], F32, tag="osb", name="o_sb")
                    nc.scalar.activation(
                        o_sb[ct][:, isl * 512:(isl + 1) * 512], ps[(ct, isl)][:],
                        IDENT, bias=total_bias[:, ct:ct + 1])
                    nc.sync.dma_start(
                        out=out_d[b, ct * 128:(ct + 1) * 128,
                                  isl * 512:(isl + 1) * 512],
                        in_=o_sb[ct][:, isl * 512:(isl + 1) * 512])
                return go

            out = []
            for ct in range(2):
                for a in range(4):
                    for isl in range(2):
                        out.append(mm(ct, isl, a))
                out += [mm_eye(ct, 0), mm_eye(ct, 1)]
            return out

        def fold_mms():
            """16 tiny closures: total_bias = b_out + b_v @ W_out."""
            bias_ps = {}

            def one(ct, kt):
                def go():
                    if ct not in bias_ps:
                        bias_ps[ct] = psC.tile([128, 2], F32, tag="C", name="bias_ps")
                    nc.tensor.matmul(out=bias_ps[ct][:],
                                     lhsT=wo8[:, kt, ct * 128:(ct + 1) * 128],
                                     rhs=zb[:, kt, :],
                                     start=(kt == 0), stop=(kt == 7))
                    if kt == 7:
                        nc.vector.tensor_add(total_bias[:, ct:ct + 1],
                                             bias_ps[ct][:, 0:1],
                                             bo_sb[:, ct:ct + 1])
                return go
            return [one(ct, kt) for ct in range(2) for kt in range(8)]

        # ---------- software-pipelined emission over units (b, h) ----------
        x8s, xbs, v8s, cats = {}, {}, {}, {}
        markers = {}

        def add_marker(key):
            flag = [False]

            def f():
                flag[0] = True
            fq.append(f)
            markers[key] = flag

        def flush_until(key):
            flag = markers.get(key)
            if flag is not None:
                while not flag[0] and fq:
                    fq.popleft()()

        def image_setup(b):
            x8s[b] = x8_pool.tile([128, 2, N], E4, tag="x8", name="x8t")
            xbs[b] = xb_pool.tile([128, 2, N], BF16, tag="xb", name="xbt")
            for isl in range(2):
                nc.scalar.copy(x8s[b][:, 0:2, isl * 512:(isl + 1) * 512],
                               xr_tiles[b][:, 0:2, isl * 512:(isl + 1) * 512])
            nc.gpsimd.tensor_copy(xbs[b][:], xr_tiles[b][:])

        image_setup(0)
        qk8s = {}
        qk8s[(0, 0)] = qk_pool.tile([128, 4, 2, 512], E4, tag="qk", name="qk8t")
        for f in qk_mms(x8s[0], qk8s[(0, 0)], 0):
            f()  # prologue: nothing to weave with yet
        for ws, h, kt in deferred_wcasts:
            nc.vector.tensor_copy(w8qk[:, kt, h * 512:(h + 1) * 512],
                                  ws[:, 0:512])
            nc.vector.tensor_copy(w8v[:, kt, h * 256:(h + 1) * 256],
                                  ws[:, 512:768])

        units = [(b, h) for b in range(B_PER_CORE) for h in range(HEADS)]
        for b, h in units:
            if h == 0:
                v8s[b] = v_pool.tile([128, 8, 1024], E4, tag="v8", name="v8t")
                cats[b] = cat_pool.tile([128, 8, 1024], E4, tag="cat", name="cat8t")
                fq.extend(v_mms(x8s[b], v8s[b]))
            nexts = {0: [1, 2], 1: [3]}.get(h, [])
            for hn in nexts:
                qk8s[(b, hn)] = qk_pool.tile([128, 4, 2, 512], E4, tag="qk", name="qk8t")
                fq.extend(qk_mms(x8s[b], qk8s[(b, hn)], hn))
                add_marker((b, hn))
            if b == 0 and h == 2:
                image_setup(1)
                qk8s[(1, 0)] = qk_pool.tile([128, 4, 2, 512], E4, tag="qk", name="qk8t")
                fq.extend(qk_mms(x8s[1], qk8s[(1, 0)], 0))
                add_marker((1, 0))

            if b == 0 and h == 1:
                # W_out + b_v staging on GPSIMD, well before the bias fold
                for kt in range(8):
                    ws2 = stage_pool.tile([128, 256], F32, tag="wostage")
                    nc.sync.dma_start(out=ws2[:],
                                      in_=wo_d[kt * 128:(kt + 1) * 128, :])
                    nc.gpsimd.tensor_copy(wo8[:, kt, :], ws2[:])
                zscr = stage_pool.tile([128, 16], F32, tag="zscr")
                nc.vector.memset(zscr[:], 0.0)
                nc.gpsimd.tensor_copy(zb[:],
                                      zscr[:].rearrange("p (a b) -> p a b", b=2))
                for kt in range(8):
                    hh, dt = kt // 2, kt % 2
                    nc.gpsimd.tensor_copy(
                        zb[:, kt, 0:1],
                        b_sb[:, hh * 6 + 4 + dt:hh * 6 + 5 + dt])
            if b == 1 and h == 0:
                fq.extend(fold_mms())
            if b == 1 and h == 1:
                fq.extend(outproj_mms(0, cats[0], xbs[0]))

            flush_until((b, h))  # qk8(b,h) drains must be emitted before scores
            qk8 = qk8s[(b, h)]
            e8 = e_pool.tile([128, 8, 1024], E4, tag="e8")
            for isl in range(2):
                for jt in range(8):
                    ps = psS.tile([128, 512], F32, tag="S")
                    nc.tensor.matmul(
                        out=ps[:],
                        lhsT=qk8[:, 2:4, jt // 4, (jt % 4) * 128:(jt % 4 + 1) * 128],
                        rhs=qk8[:, 0:2, isl, :],
                        perf_mode=DR, start=True, stop=True)
                    nc.scalar.activation(e8[:, jt, isl * 512:(isl + 1) * 512],
                                         ps[:], EXP, scale=SCALE,
                                         bias=ebias[:, 0:1])
                    fpop(3 if len(fq) > 24 else 2)
                # AV of this (h, i-half) becomes filler for what follows
                fq.extend(av_mms(e8, v8s[b], cats[b], h, isl))

        # tail: remaining AV of (b1, h3), then out projection of image 1
        fpop(len(fq))
        for f in outproj_mms(1, cats[1], xbs[1]):
            f()

    nc.compile()
    return nc


_NC = None
_EYE = np.eye(128, dtype=np.float32)


def make_in_maps(x, W_proj, b_proj, W_out, b_out):
    x = np.ascontiguousarray(x, dtype=np.float32).reshape(16, C, N)
    return [
        {
            "x": x[i * B_PER_CORE:(i + 1) * B_PER_CORE],
            "W_proj": np.ascontiguousarray(W_proj, dtype=np.float32),
            "b_proj": np.ascontiguousarray(b_proj, dtype=np.float32),
            "W_out": np.ascontiguousarray(W_out, dtype=np.float32),
            "b_out": np.ascontiguousarray(b_out, dtype=np.float32),
            "eye": _EYE,
        }
        for i in range(N_CORES)
    ]


def kernel(x, W_proj, b_proj, W_out, b_out):
    global _NC
    if _NC is None:
        _NC = _build()
    in_maps = make_in_maps(x, W_proj, b_proj, W_out, b_out)
    res = run_bass_kernel_spmd(_NC, in_maps, core_ids=list(range(N_CORES)))
    out = np.concatenate([res.results[i]["out"] for i in range(N_CORES)], axis=0)
    return out.reshape(16, C, 32, 32)


# revision 14
# speedup vs baseline: 1.1184x; 1.1184x over previous
"""Multi-head attention (B=16, C=256, N=1024, H=4 heads) on 8 TRN2 NeuronCores.

Data-parallel over batch: 2 images per core, weights replicated, no
collectives. All five GEMM stages (qkv proj, scores, softmax denominator,
AV, out proj) run in fp8 e4m3 with DoubleRow perf mode -- each matmul
contracts 256 rows (2 fp8 weights/cell) in 512 cycles, ~2x the bf16 rate.
fp32 PSUM accumulation throughout; simulated end-to-end rel err ~8e-3
(tolerance 2e-2).

Layout strategy: everything stays "transposed" ([feature, token]) so the
whole chain needs zero on-chip transposes:
  qk8[4, N]   = W_proj_slices.T @ x8    (DR: lhsT = w8qk [ci,kt,*], rhs = x8)
  attT[j, i]  = k8 @ q8.T               (DR: lhsT/rhs = qk8 slot pairs)
  E8          = exp(attT*scale - ln32)  (ScalarE, PSUM -> e4m3 SBUF direct)
  o[d, i]     = v8.T @ E8   (DR, 4 chunks of 256 j) ; s = ones8.T @ E8
  res[c, i]   = wo8.T @ cat8 (DR) + eye_bf16 @ x_bf16  (residual folded
                into the same PSUM group; drained on ScalarE with bias)

Scheduling: engines execute their streams IN ORDER, so the emission is a
software pipeline. Scores matmuls (drained by ScalarE exp at ~580ns vs
~300ns/MM production) are woven with dependency-ready "filler" matmuls
from a FIFO: v-proj, later units' qk proj, the previous unit's
AV+denominator chain, out-proj. Per-phase PSUM pools keep ring-allocation
waits from coupling phases.

DMA: per-queue bandwidth is only ~12-15 GB/s and queues come online in
index order, so startup-critical bytes (W_proj head 0, x image 0) are
issued first with >=1.5KB rows, W_proj heads 1-3 split across queues,
and the UNIT ORDER interleaves the two images --
(0,0),(0,1),(1,0),(0,2),(1,1),(0,3),(1,2),(1,3) -- so image-1 work
(x arrives early) covers the late arrival of W_proj heads 2-3.

b_proj's q/k biases are applied on the qk drains (DVE tensor_scalar
add); b_v folds through softmax (weights sum to 1) into total_bias =
b_out + b_v @ W_out via tiny fp8 matmuls, applied at the final ScalarE
drain. The identity matrix for the residual matmul rides in as an extra
DRAM input supplied by kernel() (np.eye), cast to bf16 on chip. E is
scaled by 1/32 inside the exp bias so e4m3 never saturates; the scale
cancels between numerator o and denominator s.
"""
import sys

try:
    import concourse.bass as bass  # noqa: F401
except ImportError:
    sys.path.insert(0, "/opt/trn_rl_repo")

import math
from collections import deque
from contextlib import ExitStack

import numpy as np

import concourse.bass as bass
import concourse.mybir as mybir
import concourse.tile as tile
from concourse import bacc
from concourse.bass_utils import run_bass_kernel_spmd

F32 = mybir.dt.float32
BF16 = mybir.dt.bfloat16
E4 = mybir.dt.float8e4
EXP = mybir.ActivationFunctionType.Exp
IDENT = mybir.ActivationFunctionType.Identity
DR = mybir.MatmulPerfMode.DoubleRow
MUL = mybir.AluOpType.mult

B_PER_CORE = 2   # 16 images / 8 cores
C = 256          # channels == head dim
N = 1024         # tokens (32*32)
HEADS = 4
SCALE = C ** -0.5
N_CORES = 8
NLOG32 = -math.log(32.0)

UNITS = [(0, 0), (0, 1), (1, 0), (0, 2), (1, 1), (0, 3), (1, 2), (1, 3)]


def _build():
    nc = bacc.Bacc("TRN2", debug=False, num_devices=N_CORES)
    x_d = nc.declare_dram_parameter("x", [B_PER_CORE, C, N], F32, isOutput=False)
    wp_d = nc.declare_dram_parameter("W_proj", [C, 3 * HEADS * C], F32, isOutput=False)
    bp_d = nc.declare_dram_parameter("b_proj", [3 * HEADS * C], F32, isOutput=False)
    wo_d = nc.declare_dram_parameter("W_out", [HEADS * C, C], F32, isOutput=False)
    bo_d = nc.declare_dram_parameter("b_out", [C], F32, isOutput=False)
    eye_d = nc.declare_dram_parameter("eye", [128, 128], F32, isOutput=False)
    out_d = nc.declare_dram_parameter("out", [B_PER_CORE, C, N], F32, isOutput=True)

    with tile.TileContext(nc) as tc, ExitStack() as ctx:
        pool = ctx.enter_context(tc.tile_pool(name="persist", bufs=1))
        stage_pool = ctx.enter_context(tc.tile_pool(name="stage", bufs=8))
        xr_pool = ctx.enter_context(tc.tile_pool(name="xr", bufs=2))
        x8_pool = ctx.enter_context(tc.tile_pool(name="x8", bufs=2))
        xb_pool = ctx.enter_context(tc.tile_pool(name="xb", bufs=2))
        qk_pool = ctx.enter_context(tc.tile_pool(name="qk", bufs=3))
        e_pool = ctx.enter_context(tc.tile_pool(name="e8", bufs=3))
        v_pool = ctx.enter_context(tc.tile_pool(name="v8", bufs=2))
        cat_pool = ctx.enter_context(tc.tile_pool(name="cat", bufs=2))
        r_pool = ctx.enter_context(tc.tile_pool(name="r", bufs=2))
        out_pool = ctx.enter_context(tc.tile_pool(name="outs", bufs=4))
        psS = ctx.enter_context(tc.tile_pool(name="psS", bufs=2, space="PSUM"))
        psQ = ctx.enter_context(tc.tile_pool(name="psQ", bufs=2, space="PSUM"))
        psB = ctx.enter_context(tc.tile_pool(name="psB", bufs=2, space="PSUM"))
        psC = ctx.enter_context(tc.tile_pool(name="psC", bufs=2, space="PSUM"))

        # ---- constants first (GPSIMD memsets): they gate the PE warmup ----
        ones_w = pool.tile([128, 512], BF16)
        nc.gpsimd.memset(ones_w[:], 1.0)
        ones8 = pool.tile([128, 2, 128], E4)
        nc.gpsimd.memset(ones8[:], 1.0)
        ebias = pool.tile([128, 1], F32)  # exp bias: -ln(32)
        nc.gpsimd.memset(ebias[:], NLOG32)

        # ---- DMAs: critical bytes on the earliest queues, big rows ----
        w8qk = pool.tile([128, 2, 2048], E4)
        w8v = pool.tile([128, 2, 1024], E4)
        ws_h0 = []
        for kt in range(2):                                   # q0-1
            ws = stage_pool.tile([128, 768], F32, tag="wstage")
            ws_h0.append(ws)
            nc.sync.dma_start(out=ws[:], in_=wp_d[kt * 128:(kt + 1) * 128, 0:768])
        xr = xr_pool.tile([128, 2, N], F32, tag="xr")
        for isl in range(2):                                  # q2-5
            for kt in range(2):
                nc.sync.dma_start(
                    out=xr[:, kt, isl * 512:(isl + 1) * 512],
                    in_=x_d[0, kt * 128:(kt + 1) * 128, isl * 512:(isl + 1) * 512])
        xr_tiles = [xr]
        b_sb = pool.tile([128, 24], F32)                      # q6-8
        nc.sync.dma_start(out=b_sb[:],
                          in_=bp_d[:].rearrange("(t p) -> p t", p=128))
        bo_sb = pool.tile([128, 2], F32)
        nc.sync.dma_start(out=bo_sb[:],
                          in_=bo_d[:].rearrange("(t p) -> p t", p=128))
        eye_f = stage_pool.tile([128, 128], F32, tag="eyestage")
        nc.sync.dma_start(out=eye_f[:], in_=eye_d[:, :])

        ws_rest = {}
        for kt in range(2):                                   # q9-12: head 1
            ws = stage_pool.tile([128, 768], F32, tag="wstage")
            ws_rest[(1, kt)] = ws
            for c2 in range(2):
                nc.sync.dma_start(
                    out=ws[:, c2 * 384:(c2 + 1) * 384],
                    in_=wp_d[kt * 128:(kt + 1) * 128,
                             768 + c2 * 384:768 + (c2 + 1) * 384])
        xr = xr_pool.tile([128, 2, N], F32, tag="xr")
        for kt in range(2):                                   # q13-15, q0: x img 1
            for isl in range(2):
                nc.sync.dma_start(
                    out=xr[:, kt, isl * 512:(isl + 1) * 512],
                    in_=x_d[1, kt * 128:(kt + 1) * 128, isl * 512:(isl + 1) * 512])
        xr_tiles.append(xr)
        for h in range(2, HEADS):                             # heads 2-3
            for kt in range(2):
                ws = stage_pool.tile([128, 768], F32, tag="wstage")
                ws_rest[(h, kt)] = ws
                for c2 in range(2):
                    nc.sync.dma_start(
                        out=ws[:, c2 * 384:(c2 + 1) * 384],
                        in_=wp_d[kt * 128:(kt + 1) * 128,
                                 h * 768 + c2 * 384:h * 768 + (c2 + 1) * 384])

        # head-0 + eye casts on DVE (idle this early)
        for kt in range(2):
            nc.vector.tensor_copy(w8qk[:, kt, 0:512], ws_h0[kt][:, 0:512])
            nc.vector.tensor_copy(w8v[:, kt, 0:256], ws_h0[kt][:, 512:768])
        eye_bf = pool.tile([128, 128], BF16)
        nc.vector.tensor_copy(eye_bf[:], eye_f[:])
        # heads 1-3 casts on GPSIMD (their DMA waits must not block DVE)
        for h in range(1, HEADS):
            for kt in range(2):
                ws = ws_rest[(h, kt)]
                nc.gpsimd.tensor_copy(w8qk[:, kt, h * 512:(h + 1) * 512],
                                      ws[:, 0:512])
                nc.gpsimd.tensor_copy(w8v[:, kt, h * 256:(h + 1) * 256],
                                      ws[:, 512:768])

        # dummy matmuls: fill the initial DMA wait + warm the HAM clock gate
        for wi in range(24):
            warm_ps = psS.tile([128, 512], F32, tag="S")
            nc.tensor.matmul(out=warm_ps[:], lhsT=ones_w[:, 0:128],
                             rhs=ones_w[:], start=True, stop=True)

        wo8 = pool.tile([128, 8, 256], E4)   # W_out k-tiles (loaded mid-flight)
        zb = pool.tile([128, 8, 2], E4)      # b_v columns for the bias fold
        total_bias = pool.tile([128, 2], F32)

        # ---------- emission helpers (each closure emits ~one matmul) ----------
        fq = deque()
        markers = {}

        def add_marker(key):
            flag = [False]

            def f():
                flag[0] = True
            fq.append(f)
            markers[key] = flag

        def flush_until(key):
            flag = markers.get(key)
            if flag is not None:
                while not flag[0] and fq:
                    fq.popleft()()

        def fpop(k):
            for _ in range(k):
                if fq:
                    fq.popleft()()

        def qk_mms(x8, qk8, h):
            """8 closures: q,k for head h -> qk8[128, slot, isl, 512] e4m3.
            Emission order matches scores' consumption order."""
            def one(mt, isl):
                def go():
                    ps = psQ.tile([128, 512], F32, tag="Q", name="ps_qk")
                    nc.tensor.matmul(
                        out=ps[:],
                        lhsT=w8qk[:, 0:2,
                                  h * 512 + mt * 128:h * 512 + (mt + 1) * 128],
                        rhs=x8[:, 0:2, isl * 512:(isl + 1) * 512],
                        perf_mode=DR, start=True, stop=True)
                    nc.vector.tensor_scalar_add(
                        qk8[:, mt, isl], ps[:],
                        b_sb[:, h * 6 + mt:h * 6 + mt + 1])
                return go
            order = [(0, 0), (1, 0), (2, 0), (3, 0), (2, 1), (3, 1), (0, 1), (1, 1)]
            return [one(mt, isl) for mt, isl in order]

        def v_mms(x8, v8, hp):
            """8 closures: v for heads 2hp, 2hp+1 -> v8[:, it, h*256+d]."""
            def one(it):
                def go():
                    ps = psQ.tile([128, 512], F32, tag="Q", name="ps_v")
                    nc.tensor.matmul(
                        out=ps[:],
                        lhsT=x8[:, 0:2, it * 128:(it + 1) * 128],
                        rhs=w8v[:, 0:2, hp * 512:(hp + 1) * 512],
                        perf_mode=DR, start=True, stop=True)
                    nc.vector.tensor_copy(v8[:, it, hp * 512:(hp + 1) * 512],
                                          ps[:])
                return go
            return [one(it) for it in range(8)]

        def av_mms(e8, v8, cat8, h, isl):
            """12 closures: AV + denominator for one i-half -> cat8 (normalized)."""
            o_ps = [None, None]
            s_ps = [None]

            def mm_o(a, dh):
                def go():
                    if o_ps[dh] is None:
                        o_ps[dh] = psB.tile([128, 512], F32, tag="B", name="o_ps")
                    nc.tensor.matmul(
                        out=o_ps[dh][:],
                        lhsT=v8[:, 2 * a:2 * a + 2,
                                h * 256 + dh * 128:h * 256 + (dh + 1) * 128],
                        rhs=e8[:, 2 * a:2 * a + 2, isl * 512:(isl + 1) * 512],
                        perf_mode=DR, start=(a == 0), stop=(a == 3))
                return go

            def mm_s(a):
                def go():
                    if s_ps[0] is None:
                        s_ps[0] = psC.tile([128, 512], F32, tag="C", name="s_ps")
                    nc.tensor.matmul(
                        out=s_ps[0][:], lhsT=ones8[:],
                        rhs=e8[:, 2 * a:2 * a + 2, isl * 512:(isl + 1) * 512],
                        perf_mode=DR, start=(a == 0), stop=(a == 3))
                    if a == 3:
                        r_sb = r_pool.tile([128, 512], F32, tag="r", name="r_sb")
                        nc.vector.reciprocal_approx_fast(r_sb[:], s_ps[0][:])
                        for dh2 in range(2):
                            nc.vector.scalar_tensor_tensor(
                                cat8[:, 2 * h + dh2, isl * 512:(isl + 1) * 512],
                                o_ps[dh2][:], 1.0, r_sb[:], MUL, MUL)
                return go

            out = []
            for a in range(4):
                out += [mm_o(a, 0), mm_o(a, 1), mm_s(a)]
            return out

        def outproj_mms(b, cat8, xb):
            """20 closures + drains + DMA: res[c, i] with residual + bias."""
            o_sb = [None, None]
            ps = {}

            def mm(ct, isl, a):
                def go():
                    if (ct, isl) not in ps:
                        ps[(ct, isl)] = psQ.tile([128, 512], F32, tag="Q",
                                                 name="ps_op")
                    nc.tensor.matmul(
                        out=ps[(ct, isl)][:],
                        lhsT=wo8[:, 2 * a:2 * a + 2, ct * 128:(ct + 1) * 128],
                        rhs=cat8[:, 2 * a:2 * a + 2, isl * 512:(isl + 1) * 512],
                        perf_mode=DR, start=(a == 0), stop=False)
                return go

            def mm_eye(ct, isl):
                def go():
                    nc.tensor.matmul(out=ps[(ct, isl)][:], lhsT=eye_bf[:],
                                     rhs=xb[:, ct, isl * 512:(isl + 1) * 512],
                                     start=False, stop=True)
                    if o_sb[ct] is None:
                        o_sb[ct] = out_pool.tile([128, 1024], F32, tag="osb",
                                                 name="o_sb")
                    nc.scalar.activation(
                        o_sb[ct][:, isl * 512:(isl + 1) * 512], ps[(ct, isl)][:],
                        IDENT, bias=total_bias[:, ct:ct + 1])
                    nc.sync.dma_start(
                        out=out_d[b, ct * 128:(ct + 1) * 128,
                                  isl * 512:(isl + 1) * 512],
                        in_=o_sb[ct][:, isl * 512:(isl + 1) * 512])
                return go

            out = []
            for ct in range(2):
                for a in range(4):
                    for isl in range(2):
                        out.append(mm(ct, isl, a))
                out += [mm_eye(ct, 0), mm_eye(ct, 1)]
            return out

        def fold_mms():
            """16 tiny closures: total_bias = b_out + b_v @ W_out."""
            bias_ps = {}

            def one(ct, kt):
                def go():
                    if ct not in bias_ps:
                        bias_ps[ct] = psC.tile([128, 2], F32, tag="C",
                                               name="bias_ps")
                    nc.tensor.matmul(out=bias_ps[ct][:],
                                     lhsT=wo8[:, kt, ct * 128:(ct + 1) * 128],
                                     rhs=zb[:, kt, :],
                                     start=(kt == 0), stop=(kt == 7))
                    if kt == 7:
                        nc.vector.tensor_add(total_bias[:, ct:ct + 1],
                                             bias_ps[ct][:, 0:1],
                                             bo_sb[:, ct:ct + 1])
                return go
            return [one(ct, kt) for ct in range(2) for kt in range(8)]

        # ---------- software-pipelined emission over UNITS ----------
        x8s, xbs, v8s, cats, qk8s = {}, {}, {}, {}, {}

        def image_setup(b):
            x8s[b] = x8_pool.tile([128, 2, N], E4, tag="x8", name="x8t")
            xbs[b] = xb_pool.tile([128, 2, N], BF16, tag="xb", name="xbt")
            for isl in range(2):
                nc.scalar.copy(x8s[b][:, 0:2, isl * 512:(isl + 1) * 512],
                               xr_tiles[b][:, 0:2, isl * 512:(isl + 1) * 512])

        def enqueue_qk(ui):
            b, h = UNITS[ui]
            qk8s[(b, h)] = qk_pool.tile([128, 4, 2, 512], E4, tag="qk",
                                        name="qk8t")
            fq.extend(qk_mms(x8s[b], qk8s[(b, h)], h))
            add_marker((b, h))

        image_setup(0)
        qk8s[(0, 0)] = qk_pool.tile([128, 4, 2, 512], E4, tag="qk", name="qk8t")
        for f in qk_mms(x8s[0], qk8s[(0, 0)], 0):
            f()  # prologue: nothing to weave with yet

        for ui, (b, h) in enumerate(UNITS):
            # per-unit setup / enqueues (order matters: FIFO)
            if ui == 0:
                v8s[0] = v_pool.tile([128, 8, 1024], E4, tag="v8", name="v8t")
                cats[0] = cat_pool.tile([128, 8, 1024], E4, tag="cat",
                                        name="cat8t")
                fq.extend(v_mms(x8s[0], v8s[0], 0))
                enqueue_qk(1)
            elif ui == 1:
                image_setup(1)
                v8s[1] = v_pool.tile([128, 8, 1024], E4, tag="v8", name="v8t")
                cats[1] = cat_pool.tile([128, 8, 1024], E4, tag="cat",
                                        name="cat8t")
                enqueue_qk(2)
                enqueue_qk(3)
            elif ui <= 5:
                enqueue_qk(ui + 2)

            if UNITS[ui] == (1, 0):
                fq.extend(v_mms(x8s[1], v8s[1], 0))
            elif UNITS[ui] == (0, 2):
                fq.extend(v_mms(x8s[0], v8s[0], 1))
                nc.gpsimd.tensor_copy(xbs[0][:], xr_tiles[0][:])
            elif UNITS[ui] == (1, 1):
                fq.extend(v_mms(x8s[1], v8s[1], 1))
                nc.gpsimd.tensor_copy(xbs[1][:], xr_tiles[1][:])
                fq.extend(fold_mms())
            elif UNITS[ui] == (1, 2):
                fq.extend(outproj_mms(0, cats[0], xbs[0]))

            if UNITS[ui] == (0, 1):
                # W_out + b_v staging (GPSIMD), well before the bias fold
                for kt in range(8):
                    ws2 = stage_pool.tile([128, 256], F32, tag="wostage")
                    nc.sync.dma_start(out=ws2[:],
                                      in_=wo_d[kt * 128:(kt + 1) * 128, :])
                    nc.gpsimd.tensor_copy(wo8[:, kt, :], ws2[:])
                zscr = stage_pool.tile([128, 16], F32, tag="zscr")
                nc.vector.memset(zscr[:], 0.0)
                nc.gpsimd.tensor_copy(zb[:],
                                      zscr[:].rearrange("p (a b) -> p a b", b=2))
                for kt in range(8):
                    hh, dt = kt // 2, kt % 2
                    nc.gpsimd.tensor_copy(
                        zb[:, kt, 0:1],
                        b_sb[:, hh * 6 + 4 + dt:hh * 6 + 5 + dt])

            flush_until((b, h))  # qk8(b,h) drains must be emitted before scores
            qk8 = qk8s[(b, h)]
            e8 = e_pool.tile([128, 8, 1024], E4, tag="e8")
            for isl in range(2):
                for jt in range(8):
                    ps = psS.tile([128, 512], F32, tag="S")
                    nc.tensor.matmul(
                        out=ps[:],
                        lhsT=qk8[:, 2:4, jt // 4, (jt % 4) * 128:(jt % 4 + 1) * 128],
                        rhs=qk8[:, 0:2, isl, :],
                        perf_mode=DR, start=True, stop=True)
                    nc.scalar.activation(e8[:, jt, isl * 512:(isl + 1) * 512],
                                         ps[:], EXP, scale=SCALE,
                                         bias=ebias[:, 0:1])
                    fpop(3 if len(fq) > 24 else 2)
                # AV of this (unit, i-half) becomes filler for what follows
                fq.extend(av_mms(e8, v8s[b], cats[b], h, isl))

        # tail: remaining AV of (1, 3), then out projection of image 1
        fpop(len(fq))
        for f in outproj_mms(1, cats[1], xbs[1]):
            f()

    nc.compile()
    return nc


_NC = None
_EYE = np.eye(128, dtype=np.float32)


def make_in_maps(x, W_proj, b_proj, W_out, b_out):
    x = np.ascontiguousarray(x, dtype=np.float32).reshape(16, C, N)
    return [
        {
            "x": x[i * B_PER_CORE:(i + 1) * B_PER_CORE],
            "W_proj": np.ascontiguousarray(W_proj, dtype=np.float32),
            "b_proj": np.ascontiguousarray(b_proj, dtype=np.float32),
            "W_out": np.ascontiguousarray(W_out, dtype=np.float32),
            "b_out": np.ascontiguousarray(b_out, dtype=np.float32),
            "eye": _EYE,
        }
        for i in range(N_CORES)
    ]


def kernel(x, W_proj, b_proj, W_out, b_out):
    global _NC
    if _NC is None:
        _NC = _build()
    in_maps = make_in_maps(x, W_proj, b_proj, W_out, b_out)
    res = run_bass_kernel_spmd(_NC, in_maps, core_ids=list(range(N_CORES)))
    out = np.concatenate([res.results[i]["out"] for i in range(N_CORES)], axis=0)
    return out.reshape(16, C, 32, 32)
